# revision 24
# baseline (speedup 1.0000x reference)
"""Trainium2 Bass kernel for the CSTR (evaporator) 1M-step scan.

Parallel-in-time: the per-step map is contractive (~0.965/step slow mode),
so the trajectory is split into 1024 segments (8 cores x 128 lanes) of
L=1024 steps, each extended K=384 steps back ("spin-up") so an arbitrary
segment-entry state converges below fp32 noise before the graded region.
Within each lane's window the nonlinear recurrence

  x0' = x0*(SC(u0) - c02*x0 - c03*x1) + c01
  x1' = SA(u1)*x1 + a10*x0 + SB(u0,u1)

is solved by 3 Picard-Gauss-Seidel sweeps whose linear-recurrence cores
run on the vector engine's native tensor_tensor_scan. Later sweeps start
at column 64/224: contraction washes the inherited suffix. Input DMA is
split into 2 column chunks on two DGE queues with stream precompute and
the first sweep chasing the chunks. The first K outputs are computed on
host (0.1% of the work) since segment 0 has no spin-up protection.
All param-derived scalars are per-partition [128,1] operands, so the
compiled program is input-independent.
"""

import numpy as np

T = 1048576
P = 128
NCORES = 8
L = 1024          # graded steps per lane
K = 384           # spin-up steps
W = K + L         # window length per lane (1408)
TC = T // NCORES  # steps per core
SLAB = TC + K     # u rows staged per core
NSWEEPS = 3
SWEEP_J0 = [0, 64, 224]
NQ = 2            # head chunks
NC_CONST = 17

# fixed model constants (match reference.py)
A, B, C_, D, E, F_, G, H = 0.5616, 0.3126, 48.43, 0.507, 55.0, 0.1538, 90.0, 0.16

_cache = {}


def _build_nc():
    if "nc" in _cache:
        return _cache["nc"]
    from contextlib import ExitStack
    import concourse.bacc as bacc
    import concourse.tile as tile
    import concourse.mybir as mybir
    from bass_rust import AP

    f32 = mybir.dt.float32
    op = mybir.AluOpType
    ident = mybir.ActivationFunctionType.Identity
    nc = bacc.Bacc("TRN2", target_bir_lowering=False, debug=False,
                   enable_asserts=True, num_devices=NCORES)

    uslab = nc.dram_tensor("uslab", [SLAB, 2], f32, kind="ExternalInput").ap()
    cons = nc.dram_tensor("cons", [P, NC_CONST], f32, kind="ExternalInput").ap()
    o0 = nc.dram_tensor("o0", [P, L], f32, kind="ExternalOutput").ap()
    o1 = nc.dram_tensor("o1", [P, L], f32, kind="ExternalOutput").ap()

    Wm = W - 1
    CHUNKS = [(0, 512), (512, W)]

    with tile.TileContext(nc) as tc, ExitStack() as ctx:
        pool = ctx.enter_context(tc.tile_pool(name="main", bufs=1))
        t_uwq = [pool.tile([P, 2 * (hi - lo)], f32, name=f"uw{q}", tag=f"uw{q}")
                 for q, (lo, hi) in enumerate(CHUNKS)]
        t_cons = pool.tile([P, NC_CONST], f32, tag="cons")

        def cst(i):
            return t_cons[:, i : i + 1]

        t_scr = pool.tile([P, W], f32, tag="scr")   # recip scratch
        t_rec = pool.tile([P, W], f32, tag="rec")
        t_den = pool.tile([P, W], f32, tag="den")
        t_r = pool.tile([P, W], f32, tag="r")
        t_SA = pool.tile([P, W], f32, tag="SA")
        t_SBp = pool.tile([P, W], f32, tag="SBp")
        t_SB = pool.tile([P, W], f32, tag="SB")
        t_SC = pool.tile([P, W], f32, tag="SC")
        t_b = pool.tile([P, W], f32, tag="b")
        t_v = pool.tile([P, Wm], f32, tag="v")
        t_a = pool.tile([P, Wm], f32, tag="a")
        t_c = pool.tile([P, Wm], f32, tag="c")
        t_X0 = pool.tile([P, W], f32, tag="X0")
        t_X1 = pool.tile([P, W], f32, tag="X1")

        nc.sync.dma_start(t_cons[:], cons[:])
        # warm both HWDGE queues (first-use ring init costs ~2.3us)
        nc.sync.dma_start(t_scr[0:1, 0:4], cons[0:1, 0:4])
        nc.scalar.dma_start(t_scr[0:1, 4:8], cons[0:1, 0:4])
        # ACT table warm-up (Identity) while DMA streams in
        nc.scalar.activation(t_scr[:, 0:1], t_cons[:, 0:1], ident, bias=0.0, scale=1.0)

        # input windows: 4 column chunks, each its own tile (fine-grained
        # deps so compute chases the DMA), alternating two DGE queues
        for q, (lo, hi) in enumerate(CHUNKS):
            eng = nc.sync if q % 2 == 0 else nc.scalar
            winq = AP(uslab.tensor, 2 * lo, [[L * 2, P], [1, 2 * (hi - lo)]])
            eng.dma_start(t_uwq[q][:], winq)

        nc.vector.tensor_copy(t_X0[:, 0:1], cst(15))
        nc.vector.tensor_copy(t_X1[:, 0:1], cst(16))

        # per chunk: stream precompute then the chunk's sweep-1 piece
        for q, (lo, hi) in enumerate(CHUNKS):
            wq = hi - lo
            u0q = t_uwq[q][:, 0 : 2 * wq : 2]
            u1q = t_uwq[q][:, 1 : 2 * wq : 2]
            nc.vector.tensor_scalar(t_den[:, lo:hi], u1q, cst(0), cst(1), op.mult, op.add)
            nc.vector.reciprocal_approx_fast(t_rec[:, lo:hi], t_den[:, lo:hi])
            nc.vector.scalar_tensor_tensor(t_r[:, lo:hi], u1q, cst(2),
                                           t_rec[:, lo:hi], op.mult, op.mult)
            nc.scalar.activation(t_SC[:, lo:hi], u0q, ident, bias=cst(9), scale=cst(8))
            nc.scalar.activation(t_SBp[:, lo:hi], u0q, ident, bias=cst(6), scale=cst(5))
            nc.scalar.activation(t_SA[:, lo:hi], t_r[:, lo:hi], ident,
                                 bias=cst(4), scale=cst(3))
            nc.scalar.activation(t_b[:, lo:hi], t_den[:, lo:hi], ident,
                                 bias=cst(11), scale=0.0)
            nc.vector.scalar_tensor_tensor(t_SB[:, lo:hi], t_r[:, lo:hi], cst(7),
                                           t_SBp[:, lo:hi], op.mult, op.add)
            # sweep-1 pieces for this chunk
            shi = min(hi, Wm)
            nc.vector.tensor_scalar(t_a[:, lo:shi], t_SC[:, lo:shi], cst(10),
                                    None, op.subtract)
            nc.vector.tensor_tensor_scan(t_X0[:, lo + 1 : shi + 1], t_a[:, lo:shi],
                                         t_b[:, lo:shi], t_X0[:, lo : lo + 1],
                                         op.mult, op.add)
            nc.vector.scalar_tensor_tensor(t_c[:, lo:shi], t_X0[:, lo:shi], cst(12),
                                           t_SB[:, lo:shi], op.mult, op.add)
            nc.vector.tensor_tensor_scan(t_X1[:, lo + 1 : shi + 1], t_SA[:, lo:shi],
                                         t_c[:, lo:shi], t_X1[:, lo : lo + 1],
                                         op.mult, op.add)

        # sweeps 2..N-1: full-range single ops
        for s in range(1, NSWEEPS - 1):
            j0 = SWEEP_J0[s]
            nc.vector.scalar_tensor_tensor(t_v[:, j0:Wm], t_X0[:, j0:Wm], cst(13),
                                           t_SC[:, j0:Wm], op.mult, op.add)
            nc.vector.scalar_tensor_tensor(t_a[:, j0:Wm], t_X1[:, j0:Wm], cst(14),
                                           t_v[:, j0:Wm], op.mult, op.add)
            nc.vector.tensor_tensor_scan(t_X0[:, j0 + 1 : W], t_a[:, j0:Wm],
                                         t_b[:, j0:Wm], t_X0[:, j0 : j0 + 1],
                                         op.mult, op.add)
            nc.vector.scalar_tensor_tensor(t_c[:, j0:Wm], t_X0[:, j0:Wm], cst(12),
                                           t_SB[:, j0:Wm], op.mult, op.add)
            nc.vector.tensor_tensor_scan(t_X1[:, j0 + 1 : W], t_SA[:, j0:Wm],
                                         t_c[:, j0:Wm], t_X1[:, j0 : j0 + 1],
                                         op.mult, op.add)

        # final sweep: chunked x3, output DMA inline (short tail)
        j0 = SWEEP_J0[NSWEEPS - 1]
        bounds = [(j0, 760), (760, 1180), (1180, Wm)]
        nc.vector.scalar_tensor_tensor(t_v[:, j0:Wm], t_X0[:, j0:Wm], cst(13),
                                       t_SC[:, j0:Wm], op.mult, op.add)
        nc.vector.scalar_tensor_tensor(t_a[:, j0:Wm], t_X1[:, j0:Wm], cst(14),
                                       t_v[:, j0:Wm], op.mult, op.add)
        for i, (lo, hi) in enumerate(bounds):
            nc.vector.tensor_tensor_scan(t_X0[:, lo + 1 : hi + 1], t_a[:, lo:hi],
                                         t_b[:, lo:hi], t_X0[:, lo : lo + 1],
                                         op.mult, op.add)
            nc.vector.scalar_tensor_tensor(t_c[:, lo:hi], t_X0[:, lo:hi], cst(12),
                                           t_SB[:, lo:hi], op.mult, op.add)
            dlo, dhi = max(lo + 1, K), hi + 1
            if dhi > dlo:
                nc.sync.dma_start(o0[:, dlo - K : dhi - K], t_X0[:, dlo:dhi])
            nc.vector.tensor_tensor_scan(t_X1[:, lo + 1 : hi + 1], t_SA[:, lo:hi],
                                         t_c[:, lo:hi], t_X1[:, lo : lo + 1],
                                         op.mult, op.add)
            if dhi > dlo:
                nc.scalar.dma_start(o1[:, dlo - K : dhi - K], t_X1[:, dlo:dhi])

    nc.compile()
    _cache["nc"] = nc
    return nc


def _derive(params, x0):
    M, Cc, UA2, Cp, lam, lams, F1, X1p, F3, T1, T200 = [float(params[i]) for i in range(11)]
    UA1 = H * (F1 + F3)
    k1 = (UA1 + F1 * Cp) / lam
    p_ = k1 * B
    q_ = k1 * A
    alpha_u = UA1 * F_ / lam
    alpha_c = (UA1 * G + F1 * Cp * T1) / lam - k1 * C_
    c01 = F1 * X1p / M
    c02 = p_ / M
    c03 = q_ / M
    a10 = -p_ / Cc
    i0, i1 = float(x0[0]), float(x0[1])

    cv = np.zeros(NC_CONST, np.float64)
    cv[0] = 2.0 * Cp
    cv[1] = UA2
    cv[2] = 2.0 * Cp * UA2
    cv[3] = -D / (lam * Cc)               # cA2
    cv[4] = 1.0 - q_ / Cc                 # cA1
    cv[5] = alpha_u / Cc                  # cB2
    cv[6] = alpha_c / Cc                  # cB1
    cv[7] = -(E - T200) / (lam * Cc)      # cB3
    cv[8] = alpha_u / M                   # cC2
    cv[9] = 1.0 - (F1 - alpha_c) / M      # cC1
    cv[10] = c02 * i0 + c03 * i1          # sweep-1 a offset
    cv[11] = c01                          # scan0 additive const
    cv[12] = a10                          # c coefficient
    cv[13] = -c02
    cv[14] = -c03
    cv[15] = i0
    cv[16] = i1
    return cv.astype(np.float32)


def _make_in_maps(u, x0, params):
    u = np.ascontiguousarray(u, np.float32)
    cons = np.tile(_derive(params, x0)[None, :], (P, 1))
    in_maps = []
    for c in range(NCORES):
        if c == 0:
            slab = np.concatenate([np.repeat(u[0:1], K, axis=0), u[0:TC]], axis=0)
        else:
            slab = u[c * TC - K : c * TC + TC]
        in_maps.append({
            "uslab": np.ascontiguousarray(slab),
            "cons": cons,
        })
    return in_maps


def _host_head(u, x0, params, n):
    # exact fp32 simulation of the first n steps (segment 0 has no spin-up)
    f = np.float32
    M, Cc, UA2, Cp, lam, lams, F1, X1p, F3, T1, T200 = [f(params[i]) for i in range(11)]
    out = np.empty((n, 2), f)
    s0, s1 = f(x0[0]), f(x0[1])
    fA, fB, fC, fD, fE, fF, fG, fH = f(A), f(B), f(C_), f(D), f(E), f(F_), f(G), f(H)
    one, two = f(1.0), f(2.0)
    UA1 = fH * (F1 + F3)
    for t in range(n):
        out[t, 0] = s0
        out[t, 1] = s1
        u0, u1 = f(u[t, 0]), f(u[t, 1])
        T2 = fA * s1 + fB * s0 + fC
        T3 = fD * s1 + fE
        T100 = fF * u0 + fG
        Q100 = UA1 * (T100 - T2)
        Q200 = UA2 * (T3 - T200) / (one + UA2 / (two * Cp * u1))
        F5 = Q200 / lam
        F4 = (Q100 - F1 * Cp * (T2 - T1)) / lam
        F2 = F1 - F4
        X2d = (F1 * X1p - F2 * s0) / M
        P2d = (F4 - F5) / Cc
        s0 = s0 + X2d
        s1 = s1 + P2d
    return out


def _assemble(results, head):
    out = np.empty((T, 2), np.float32)
    for c in range(NCORES):
        out[c * TC : (c + 1) * TC, 0] = results[c]["o0"].reshape(-1)
        out[c * TC : (c + 1) * TC, 1] = results[c]["o1"].reshape(-1)
    out[0:L] = head
    return out


def run(u_forced, x0, params, trace=False):
    from concourse.bass_utils import run_bass_kernel_spmd
    nc = _build_nc()
    in_maps = _make_in_maps(u_forced, x0, params)
    head = _host_head(u_forced, x0, params, L)
    res = run_bass_kernel_spmd(nc, in_maps, list(range(NCORES)), trace=trace)
    return _assemble(res.results, head), res


def kernel(u_forced, x0, params):
    out, _ = run(u_forced, x0, params, trace=False)
    return out
